# revision 6
# baseline (speedup 1.0000x reference)
"""Trainium2 Bass kernel for a GNN message-passing layer.

Strategy (node-range sharding, host-side gather, no collectives):
  - 8 cores, each owns 12500 destination nodes (98 windows of 128 nodes).
  - Host sorts edges by (core, dst-window), pads each window to 128-edge
    tiles (tile counts = max over cores so one NEFF fits all cores), and
    pre-gathers x[src], x[dst] into an edge-major bf16 stream
    xsd[128, e_pad] (rows 0:64 = x[src]^T, rows 64:128 = x[dst]^T) plus
    attrA[17, e_pad] (edge_attr^T with a constant ones row for bias fold).
  - Device, per 512-edge chunk: h^T = W1^T m_in via 2 wide matmuls
    (K=128 for [xsrc;xdst], K=17 for [attr;1]); b1 folded into the attr
    weights; an extra output column makes silu produce a constant 1.0 row
    so b2 folds into the msg matmul. Then per 128-edge tile: msg
    edge-major via lhsT=h-slice, scatter-add to agg[64, 128n] in PSUM via
    a one-hot sel matmul. sel for the whole chunk is built in one DVE
    is_equal with a stride-0 broadcast AP.
  - Node MLP per window: out^T[64, 128n] = W3^T [x_win; agg] with b3 via
    the activation bias port; output written feat-major, host transposes.

All matmuls bf16 (f32 PSUM accumulate).
"""

import numpy as np
import ml_dtypes

P = 128
H = 64
ED = 16
N_CORES = 8
CHUNK = 4          # tiles per chunk (4*128 = 512 edges, one PSUM bank)


# ---------------------------------------------------------------- host prep

def _silu_inv_one():
    """z with z*sigmoid(z) == 1 (float64 Newton)."""
    z = 1.3
    for _ in range(50):
        s = 1.0 / (1.0 + np.exp(-z))
        f = z * s - 1.0
        df = s * (1.0 + z * (1.0 - s))
        z -= f / df
    return z


def _prep(x, edge_index, edge_attr):
    """Sort/pad edges into per-core slot layout; host-gather x[src]/x[dst]."""
    bf16 = ml_dtypes.bfloat16
    n_nodes = x.shape[0]
    npc = n_nodes // N_CORES              # nodes per core
    nw = (npc + P - 1) // P               # windows per core
    npc_pad = nw * P

    src = edge_index[0].astype(np.int64)
    dst = edge_index[1].astype(np.int64)
    e = src.shape[0]

    core = dst // npc
    rem = dst - core * npc
    wl = rem // P                         # window within core
    dloc = rem - wl * P                   # dst index within window

    key = (core * nw + wl).astype(np.int64)
    order = np.argsort(key, kind="stable")
    key_s = key[order]
    src_s = src[order]
    dst_s = dst[order]
    dloc_s = dloc[order]

    counts = np.bincount(key_s, minlength=N_CORES * nw).reshape(N_CORES, nw)
    tw = np.maximum((counts.max(axis=0) + P - 1) // P, 1)   # tiles per window
    sw = tw * P                                             # slots per window
    e_pad = int(sw.sum())
    t_tot = int(tw.sum())
    base = np.concatenate([[0], np.cumsum(sw)[:-1]])        # slot base per w

    # rank of each sorted edge within its (core, w) block
    starts = np.concatenate([[0], np.cumsum(counts.reshape(-1))[:-1]])
    rank = np.arange(e, dtype=np.int64) - starts[key_s]
    core_s = key_s // nw
    w_s = key_s - core_s * nw
    slot = base[w_s] + rank

    # edge-major streams (pad cols stay 0 / dloc -1)
    xsd = np.zeros((N_CORES, 2 * H, e_pad), dtype=bf16)
    xb = x.astype(bf16)
    xsd[core_s, :, slot] = np.concatenate([xb[src_s], xb[dst_s]], axis=1)

    attrA = np.zeros((N_CORES, ED + 1, e_pad), dtype=bf16)
    attrA[:, ED, :] = bf16(1.0)
    attrA[core_s, :, slot] = np.concatenate(
        [edge_attr[order].astype(bf16),
         np.ones((e, 1), dtype=bf16)], axis=1)

    dloc_slots = np.full((N_CORES, e_pad), -1.0, dtype=np.float32)
    dloc_slots[core_s, slot] = dloc_s.astype(np.float32)
    dstc = np.ascontiguousarray(
        dloc_slots.reshape(N_CORES, t_tot, P).transpose(0, 2, 1)
    ).astype(bf16)                                          # [C, 128, Ttot]

    xT = x.T.astype(bf16)                                   # [64, N]
    xTn = np.zeros((N_CORES, H, npc_pad), dtype=bf16)
    for c in range(N_CORES):
        xTn[c, :, :npc] = xT[:, c * npc:(c + 1) * npc]

    struct = {"nw": nw, "npc": npc, "npc_pad": npc_pad, "e_pad": e_pad,
              "t_tot": t_tot, "tw": tw, "sw": sw}
    arrays = {"xsd": xsd, "attrA": attrA, "dstc": dstc, "xTn": xTn}
    return struct, arrays


def _prep_consts(W1, b1, W2, b2, W3, b3):
    bf16 = ml_dtypes.bfloat16
    z1 = _silu_inv_one()

    w1ab = np.zeros((2 * H, H + 1), dtype=bf16)
    w1ab[:, :H] = W1[0:2 * H, :].astype(bf16)

    w1ca = np.zeros((ED + 1, H + 1), dtype=bf16)
    w1ca[0:ED, :H] = W1[2 * H:2 * H + ED, :].astype(bf16)
    w1ca[ED, :H] = b1.astype(bf16)
    w1ca[ED, H] = bf16(z1)          # silu -> exact-ish 1.0 constant row

    w2a = np.zeros((H + 1, H), dtype=bf16)
    w2a[0:H, :] = W2.astype(bf16)
    w2a[H, :] = b2.astype(bf16)

    iorat4 = np.broadcast_to(
        np.tile(np.arange(P, dtype=np.float32), CHUNK), (P, CHUNK * P)
    ).copy().astype(bf16)

    consts = {
        "w1ab": w1ab,
        "w1ca": w1ca,
        "w2a": w2a,
        "w3": W3.astype(bf16),
        "b3c": b3.reshape(H, 1).astype(np.float32),
        "iorat4": iorat4,
        "zeros64": np.zeros((H, P), dtype=bf16),
    }
    return consts


# ---------------------------------------------------------------- device IR

def _build(struct):
    import concourse.mybir as mybir
    import concourse.tile as tile
    from concourse import bacc

    nw = struct["nw"]
    npc_pad = struct["npc_pad"]
    e_pad = struct["e_pad"]
    t_tot = struct["t_tot"]
    tw = struct["tw"]

    bf = mybir.dt.bfloat16
    f32 = mybir.dt.float32
    AF = mybir.ActivationFunctionType
    ALU = mybir.AluOpType

    nc = bacc.Bacc("TRN2", target_bir_lowering=False)

    xsd = nc.dram_tensor("xsd", [2 * H, e_pad], bf, kind="ExternalInput")
    attrA = nc.dram_tensor("attrA", [ED + 1, e_pad], bf, kind="ExternalInput")
    dstc = nc.dram_tensor("dstc", [P, t_tot], bf, kind="ExternalInput")
    xTn = nc.dram_tensor("xTn", [H, npc_pad], bf, kind="ExternalInput")
    w1ab = nc.dram_tensor("w1ab", [2 * H, H + 1], bf, kind="ExternalInput")
    w1ca = nc.dram_tensor("w1ca", [ED + 1, H + 1], bf, kind="ExternalInput")
    w2a = nc.dram_tensor("w2a", [H + 1, H], bf, kind="ExternalInput")
    w3 = nc.dram_tensor("w3", [2 * H, H], bf, kind="ExternalInput")
    b3c = nc.dram_tensor("b3c", [H, 1], f32, kind="ExternalInput")
    iorat4 = nc.dram_tensor("iorat4", [P, CHUNK * P], bf, kind="ExternalInput")
    zeros64 = nc.dram_tensor("zeros64", [H, P], bf, kind="ExternalInput")
    outT = nc.dram_tensor("outT", [H, npc_pad], f32, kind="ExternalOutput")

    with tile.TileContext(nc) as tc:
        with (
            tc.tile_pool(name="const", bufs=1) as cp,
            tc.tile_pool(name="win", bufs=2) as wp,
            tc.tile_pool(name="work", bufs=3) as kp,
            tc.tile_pool(name="nodein", bufs=2) as np_,
            tc.tile_pool(name="outp", bufs=2) as op_,
            tc.tile_pool(name="ps_h", bufs=2, space="PSUM") as ph,
            tc.tile_pool(name="ps_m", bufs=2, space="PSUM") as pm,
            tc.tile_pool(name="ps_a", bufs=2, space="PSUM") as pa,
            tc.tile_pool(name="ps_x", bufs=2, space="PSUM") as px,
        ):
            def load_const(t, shape, dt):
                s = cp.tile(shape, dt, tag=t.name)
                nc.sync.dma_start(out=s[:], in_=t[:])
                return s

            w1abt = load_const(w1ab, [2 * H, H + 1], bf)
            w1cat = load_const(w1ca, [ED + 1, H + 1], bf)
            w2at = load_const(w2a, [H + 1, H], bf)
            w3t = load_const(w3, [2 * H, H], bf)
            b3t = load_const(b3c, [H, 1], f32)
            iot = load_const(iorat4, [P, CHUNK * P], bf)
            zt = load_const(zeros64, [H, P], bf)

            # flat chunk schedule: (w, c0, tpc, first, last)
            base = np.concatenate([[0], np.cumsum(tw * P)[:-1]]).astype(int)
            baseT = np.concatenate([[0], np.cumsum(tw)[:-1]]).astype(int)
            chunks = []
            for w in range(nw):
                t_w = int(tw[w])
                for c0 in range(0, t_w, CHUNK):
                    tpc = min(CHUNK, t_w - c0)
                    chunks.append((w, c0, tpc, c0 == 0,
                                   c0 + tpc == t_w))

            wtiles = {}   # per-window SBUF tiles
            wpsum = {}    # per-window agg PSUM
            ctiles = {}   # per-chunk tiles

            def emit_dma(w):
                t_w = int(tw[w])
                s_w = t_w * P
                col = int(base[w])
                colT = int(baseT[w])
                tA = wp.tile([2 * H, s_w], bf, tag="tA")
                nc.sync.dma_start(out=tA[:], in_=xsd[:, col:col + s_w])
                tB = wp.tile([ED + 1, s_w], bf, tag="tB")
                nc.sync.dma_start(out=tB[:], in_=attrA[:, col:col + s_w])
                dct = wp.tile([P, t_w], bf, tag="dct")
                nc.sync.dma_start(out=dct[:], in_=dstc[:, colT:colT + t_w])
                nit = np_.tile([P, P], bf, tag="nit")
                nc.sync.dma_start(out=nit[0:H, :],
                                  in_=xTn[:, w * P:(w + 1) * P])
                wtiles[w] = (tA, tB, dct, nit)

            def emit_h(k):
                w, c0, tpc, first, _ = chunks[k]
                tA, tB, dct, _ = wtiles[w]
                if first:
                    wpsum[w] = pa.tile([H, P], f32, tag="agg", name="aggps")
                cw = tpc * P
                cols = slice(c0 * P, c0 * P + cw)
                hps = ph.tile([H + 1, CHUNK * P], f32, tag="hps")
                nc.tensor.matmul(hps[:, :cw], lhsT=w1abt[:],
                                 rhs=tA[:, cols],
                                 start=True, stop=False,
                                 skip_group_check=True)
                nc.tensor.matmul(hps[:, :cw], lhsT=w1cat[:],
                                 rhs=tB[:, cols],
                                 start=False, stop=True,
                                 skip_group_check=True)
                hsb = kp.tile([H + 1, CHUNK * P], bf, tag="hsb")
                nc.scalar.activation(hsb[:, :cw], hps[:, :cw], AF.Silu)
                # one-hot sel for the whole chunk:
                # sel[p, t, n] = (dloc[tile t, edge p] == n)
                selc = kp.tile([P, CHUNK * P], bf, tag="selc")
                nc.vector.tensor_tensor(
                    out=selc[:, :cw].rearrange("p (c o) -> p c o", o=P),
                    in0=dct[:, c0:c0 + tpc]
                        .rearrange("p (c o) -> p c o", o=1)
                        .to_broadcast([P, tpc, P]),
                    in1=iot[:, :cw].rearrange("p (c o) -> p c o", o=P),
                    op=ALU.is_equal,
                )
                ctiles[k] = (hsb, selc)

            def emit_msg(k):
                _, _, tpc, _, _ = chunks[k]
                hsb, _ = ctiles[k]
                msgps = pm.tile([P, CHUNK * H], f32, tag="msgps")
                for t in range(tpc):
                    nc.tensor.matmul(
                        msgps[:, t * H:(t + 1) * H],
                        lhsT=hsb[:, t * P:(t + 1) * P],
                        rhs=w2at[:],
                        start=True, stop=True, skip_group_check=True)
                msgt = kp.tile([P, CHUNK * H], bf, tag="msgt")
                nc.scalar.activation(msgt[:, :tpc * H],
                                     msgps[:, :tpc * H], AF.Silu)
                ctiles[k] = (ctiles[k][1], msgt)   # (selc, msgt)

            def emit_scatter(k):
                w, c0, tpc, _, _ = chunks[k]
                selc, msgt = ctiles.pop(k)
                t_w = int(tw[w])
                aggps = wpsum[w]
                for t in range(tpc):
                    tt = c0 + t
                    nc.tensor.matmul(
                        aggps[:],
                        lhsT=msgt[:, t * H:(t + 1) * H],
                        rhs=selc[:, t * P:(t + 1) * P],
                        start=(tt == 0), stop=(tt == t_w - 1),
                        skip_group_check=True)

            def emit_tail(w):
                # node MLP (feat-major): out = silu(W3^T [x_win; agg] + b3)
                nit = wtiles.pop(w)[3]
                aggps = wpsum.pop(w)
                nc.vector.tensor_copy(out=nit[H:2 * H, :], in_=aggps[:])
                ops = px.tile([H, P], f32, tag="ops")
                nc.tensor.matmul(ops[:], lhsT=w3t[:], rhs=nit[:],
                                 start=True, stop=True, skip_group_check=True)
                oo = op_.tile([H, P], f32, tag="oo")
                nc.scalar.activation(oo[:], ops[:], AF.Silu, bias=b3t[:])
                nc.sync.dma_start(out=outT[:, w * P:(w + 1) * P], in_=oo[:])

            # software-pipelined emission: h-matmuls run one chunk ahead
            # of msg/scatter so PE stays busy during silu on Scalar.
            emit_dma(0)
            for k, ch in enumerate(chunks):
                w, _, _, first, _ = ch
                if first and w + 1 < nw:
                    emit_dma(w + 1)
                if k > 0:
                    emit_msg(k - 1)
                emit_h(k)
                if k > 0:
                    emit_scatter(k - 1)
                    if chunks[k - 1][4]:
                        emit_tail(chunks[k - 1][0])
            emit_msg(len(chunks) - 1)
            emit_scatter(len(chunks) - 1)
            emit_tail(chunks[-1][0])

    nc.compile()
    return nc


# ---------------------------------------------------------------- entry

def kernel(x, edge_index, edge_attr, W1, b1, W2, b2, W3, b3):
    import time
    t0 = time.time()
    x = np.asarray(x, dtype=np.float32)
    edge_index = np.asarray(edge_index)
    edge_attr = np.asarray(edge_attr, dtype=np.float32)

    struct, arrays = _prep(x, edge_index, edge_attr)
    consts = _prep_consts(
        np.asarray(W1, np.float32), np.asarray(b1, np.float32),
        np.asarray(W2, np.float32), np.asarray(b2, np.float32),
        np.asarray(W3, np.float32), np.asarray(b3, np.float32))
    t1 = time.time()

    nc = _build(struct)
    t2 = time.time()
    print(f"[kernel] prep {t1 - t0:.1f}s  build+tile {t2 - t1:.1f}s")

    from concourse.bass_utils import run_bass_kernel_spmd
    in_maps = []
    for c in range(N_CORES):
        m = {
            "xsd": arrays["xsd"][c], "attrA": arrays["attrA"][c],
            "dstc": arrays["dstc"][c], "xTn": arrays["xTn"][c],
        }
        m.update(consts)
        in_maps.append(m)
    t3 = time.time()
    res = run_bass_kernel_spmd(nc, in_maps, core_ids=list(range(N_CORES)))
    print(f"[kernel] compile+run {time.time() - t3:.1f}s")
    npc = struct["npc"]
    pieces = [np.ascontiguousarray(res.results[c]["outT"][:, :npc].T)
              for c in range(N_CORES)]
    return np.concatenate(pieces, axis=0).astype(np.float32)
